# revision 12
# baseline (speedup 1.0000x reference)
"""Distributed Trainium2 Bass kernel for a 2-layer GCN + readout.

Reference computation:
    src,dst += self loops; deg = indegree; dinv = rsqrt(deg)
    h1 = relu((dinv*(A01+I)@(dinv*x)) @ W1 + b1)
    h2 = relu((dinv*(A01+I)@(dinv*h1)) @ W2 + b2)
    out = h2.reshape(n//16, 16*256) @ Wout + bout

Strategy (8 NeuronCores, SPMD):
  - nodes sharded by contiguous range; each core aggregates the edges whose
    dst lands in its shard (host does the index bucketing / padding only).
  - gather tables (dinv*x, dinv*h1, bf16) are AllGather'd in FOUR quarter
    pieces each so the collective streams while compute proceeds.
  - the Q7 descriptor generation for dma_gather is the bottleneck engine, so
    the kernel is organized to keep 4 gather instructions in flight:
    supergroups of 8 dst windows iterate chunk-major, issuing 4 independent
    2-window gathers per chunk phase striped over the 4 SWDGE queues.
  - aggregation per 128-edge batch: one-hot selection matrix (DVE is_equal of
    dst-local ids vs an iota row) x gathered messages on the PE; per-(window,
    chunk) partials land in transient PSUM tiles and are accumulated into
    SBUF f32 accumulators by the DVE, freeing PSUM banks for deep pipelining.
  - self-loop edges are never gathered: the accumulator is initialized with
    the core's own table rows (the self-loop term) read by plain DMA.
  - trailing pad indices are -1 so the Q7 skips them; slab buffers are
    memset-primed once per layer so skipped rows stay finite (0 x finite).
  - dense transforms/readout in fp32 on PE via PE-transpose.
"""
import numpy as np
import ml_dtypes

import concourse.bass as bass
import concourse.bacc as bacc
import concourse.mybir as mybir
import concourse.tile as tile
from concourse import bass_utils
from concourse.masks import make_identity

BF16 = ml_dtypes.bfloat16
F32 = mybir.dt.float32
MBF16 = mybir.dt.bfloat16
I16 = mybir.dt.int16
P = 128


def _cfg_full():
    return dict(n=131072, e=4194304, d_in=128, d_hid=256, ncores=8,
                group=2, sgg=4, nquarter=4, sub=16)


def _prep(x, edge_index, cfg):
    """Host-side index preprocessing.

    Chunk q of the gather table = the union over cores of each core's q-th
    quarter-shard (rows core*nsq + r), i.e. the output of one quarter
    AllGather. gidx addresses rows inside a chunk. Self-loops are excluded
    (handled by accumulator init from the local shard).
    """
    n, ncores = cfg["n"], cfg["ncores"]
    ns = n // ncores
    nwin = ns // P
    nq = cfg["nquarter"]
    nsq = ns // nq               # rows per quarter shard (4096)
    nchunks = nq
    csize = nsq * ncores         # rows per chunk table (32768)

    src = np.asarray(edge_index[0], dtype=np.int64).astype(np.int32)
    dst = np.asarray(edge_index[1], dtype=np.int64).astype(np.int32)

    # degree includes self-loops (reference adds them)
    deg = (np.bincount(dst, minlength=n) + 1).astype(np.float32)

    # drop self-loop edges from the random edge list too (they are part of
    # the aggregation; the loop term itself is added via accumulator init,
    # but an explicit random edge i->i must still be aggregated)
    core_of = dst // ns
    per_core = []
    counts = np.zeros((ncores, nwin, nchunks), dtype=np.int64)
    src_chunk = (src % ns) // nsq
    src_off = (src // ns) * nsq + (src % nsq)   # position within chunk table
    for c in range(ncores):
        m = core_of == c
        s_c = src_off[m]
        d_c = dst[m] - c * ns
        ch_c = src_chunk[m]
        w_c = d_c >> 7
        order = np.lexsort((ch_c, w_c))
        s_c, d_c, ch_c = s_c[order], d_c[order], ch_c[order]
        w_c = w_c[order]
        key = w_c * nchunks + ch_c
        counts[c] = np.bincount(key, minlength=nwin * nchunks).reshape(nwin, nchunks)
        per_core.append((s_c, d_c, key))

    # static per (window, chunk) section sizes: max over cores, multiple of 128
    S = np.zeros((nwin, nchunks), dtype=np.int64)
    mx = counts.max(axis=0)
    S[mx > 0] = ((mx[mx > 0] + P - 1) // P) * P

    G, SGG = cfg["group"], cfg["sgg"]
    ngroups = (nwin + G - 1) // G
    nsg = (ngroups + SGG - 1) // SGG
    # idx iteration order must match the gather issue order in _build:
    # supergroup -> chunk -> group -> window.
    idx_order = [(w, ch)
                 for t in range(nsg)
                 for ch in range(nchunks)
                 for g in range(t * SGG, min((t + 1) * SGG, ngroups))
                 for w in range(g * G, min((g + 1) * G, nwin))
                 if S[w, ch] > 0]
    dst_order = [(w, ch)
                 for w in range(nwin)
                 for ch in range(nchunks)
                 if S[w, ch] > 0]

    idx_maps, dst_maps = [], []
    for c in range(ncores):
        s_c, d_c, key = per_core[c]
        starts = np.zeros(nwin * nchunks + 1, dtype=np.int64)
        np.cumsum(np.bincount(key, minlength=nwin * nchunks), out=starts[1:])
        idx_cols, dst_cols = {}, {}
        for w in range(nwin):
            for ch in range(nchunks):
                s_wch = int(S[w, ch])
                if s_wch == 0:
                    continue
                a, b = starts[w * nchunks + ch], starts[w * nchunks + ch + 1]
                # trailing pad: -1 on the group's last window so the Q7 trims
                # it (slabs are memset-primed, so skipped rows read 0.0)
                padv = 0  # (-1 trailing-trim caused device faults; keep 0)
                gidx = np.full(s_wch, padv, dtype=np.int16)
                gdst = np.full(s_wch, 255, dtype=np.float32)
                cnt = b - a
                gidx[:cnt] = s_c[a:b].astype(np.int16)
                gdst[:cnt] = (d_c[a:b] - w * P).astype(np.float32)
                # gather idx layout: pos i -> [i%16, i//16], replicated to 128p
                wrap = gidx.reshape(-1, 16).T  # [16, s/16]
                idx_cols[(w, ch)] = np.tile(wrap, (8, 1))
                # dst-local layout: pos i -> [i%128, i//128]
                dst_cols[(w, ch)] = gdst.reshape(-1, P).T.astype(BF16)
        idx_maps.append(idx_cols)
        dst_maps.append(dst_cols)

    idx_off, dst_off = {}, {}
    off = 0
    for w, ch in idx_order:
        idx_off[(w, ch)] = off
        off += int(S[w, ch]) // 16
    idx_total = off
    off = 0
    for w, ch in dst_order:
        dst_off[(w, ch)] = off
        off += int(S[w, ch]) // P
    dst_total = off

    idx_up = np.zeros((ncores, P, max(idx_total, 1)), dtype=np.int16)
    dst_up = np.full((ncores, P, max(dst_total, 1)), 255, dtype=BF16)
    for c in range(ncores):
        for (w, ch), arr in idx_maps[c].items():
            o = idx_off[(w, ch)]
            idx_up[c, :, o:o + arr.shape[1]] = arr
        for (w, ch), arr in dst_maps[c].items():
            o = dst_off[(w, ch)]
            dst_up[c, :, o:o + arr.shape[1]] = arr

    deg_up = np.stack([
        deg[c * ns:(c + 1) * ns].reshape(nwin, P).T for c in range(ncores)
    ])  # [ncores, 128, nwin]

    # max gather width (columns of a [P, *, d] slab) per chunk-group section
    maxblk = 0
    for t in range(nsg):
        for ch in range(nchunks):
            for g in range(t * SGG, min((t + 1) * SGG, ngroups)):
                sg = sum(int(S[w, ch])
                         for w in range(g * G, min((g + 1) * G, nwin)))
                maxblk = max(maxblk, sg // P)

    meta = dict(S=S, idx_off=idx_off, dst_off=dst_off,
                idx_total=max(idx_total, 1), dst_total=max(dst_total, 1),
                nwin=nwin, nchunks=nchunks, ns=ns, csize=csize, nsq=nsq,
                maxblk=maxblk)
    return meta, idx_up, dst_up, deg_up


def _build(cfg, meta):
    n, ncores = cfg["n"], cfg["ncores"]
    d_in, d_hid, sub = cfg["d_in"], cfg["d_hid"], cfg["sub"]
    G, SGG, nq = cfg["group"], cfg["sgg"], cfg["nquarter"]
    ns, nwin, nchunks = meta["ns"], meta["nwin"], meta["nchunks"]
    csize, nsq = meta["csize"], meta["nsq"]
    S, idx_off, dst_off = meta["S"], meta["idx_off"], meta["dst_off"]
    maxblk = meta["maxblk"]
    rpw = P // sub  # out rows per window
    ngroups = (nwin + G - 1) // G
    nsg = (ngroups + SGG - 1) // SGG
    wq = nwin // nq              # windows per quarter (32)
    sgq = nsg // nq              # supergroups per quarter
    rg = [list(range(ncores))]

    nc = bacc.Bacc("TRN2", target_bir_lowering=False, debug=False,
                   num_devices=ncores, num_swdge_queues=4)

    x_in = nc.dram_tensor("x", [ns, d_in], F32, kind="ExternalInput")
    deg_in = nc.dram_tensor("deg", [P, nwin], F32, kind="ExternalInput")
    w1_in = nc.dram_tensor("W1", [d_in, d_hid], F32, kind="ExternalInput")
    b1_in = nc.dram_tensor("b1", [d_hid], F32, kind="ExternalInput")
    w2_in = nc.dram_tensor("W2", [d_hid, d_hid], F32, kind="ExternalInput")
    b2_in = nc.dram_tensor("b2", [d_hid], F32, kind="ExternalInput")
    woutr_in = nc.dram_tensor("WoutR", [sub, d_hid], F32, kind="ExternalInput")
    bout_in = nc.dram_tensor("bout", [1], F32, kind="ExternalInput")
    u_in = nc.dram_tensor("U", [P, rpw], F32, kind="ExternalInput")
    iota_in = nc.dram_tensor("iota", [P, P], MBF16, kind="ExternalInput")
    idx_in = nc.dram_tensor("idx", [P, meta["idx_total"]], I16, kind="ExternalInput")
    dstl_in = nc.dram_tensor("dstl", [P, meta["dst_total"]], MBF16, kind="ExternalInput")
    out = nc.dram_tensor("out", [ns // sub, 1], F32, kind="ExternalOutput")

    with tile.TileContext(nc) as tc:
        with tc.tile_pool(name="dram", bufs=1, space="DRAM") as dram, \
             tc.tile_pool(name="const", bufs=1) as const:

            # quarter shards (collective inputs) and chunk tables (outputs)
            xs_sh4 = [dram.tile([nsq, d_in], MBF16, name=f"xs_sh{q}")
                      for q in range(nq)]
            xs_q = [dram.tile([csize, d_in], MBF16, name=f"xs_q{q}")
                    for q in range(nq)]
            g1_sh4 = [dram.tile([nsq, d_hid], MBF16, name=f"g1_sh{q}")
                      for q in range(nq)]
            g1_q = [dram.tile([csize, d_hid], MBF16, name=f"g1_q{q}")
                    for q in range(nq)]

            # ---- constants / weights in SBUF ----
            identity = const.tile([P, P], F32)
            make_identity(nc, identity[:])
            iota_sb = const.tile([P, P], MBF16)
            nc.sync.dma_start(iota_sb[:], iota_in[:])
            w1_sb = const.tile([P, d_hid], F32, tag="w1")
            nc.sync.dma_start(w1_sb[:], w1_in[:])
            w2_sb = [const.tile([P, d_hid], F32, tag=f"w2_{k}", name=f"w2_{k}")
                     for k in range(d_hid // P)]
            for k in range(d_hid // P):
                nc.sync.dma_start(w2_sb[k][:], w2_in[k * P:(k + 1) * P, :])
            b1rep = const.tile([P, d_hid], F32, tag="b1rep")
            nc.sync.dma_start(b1rep[:], b1_in[None, :].to_broadcast([P, d_hid]))
            b2rep = const.tile([P, d_hid], F32, tag="b2rep")
            nc.sync.dma_start(b2rep[:], b2_in[None, :].to_broadcast([P, d_hid]))
            wrep = const.tile([P, d_hid], F32, tag="wrep")
            nc.sync.dma_start(
                wrep[:], woutr_in[None, :, :].to_broadcast([P // sub, sub, d_hid]))
            u_sb = const.tile([P, rpw], F32, tag="u")
            nc.sync.dma_start(u_sb[:], u_in[:])
            boutrep = const.tile([rpw, 1], F32, tag="bout")
            nc.sync.dma_start(boutrep[:], bout_in[None, :].to_broadcast([rpw, 1]))
            out_stage = const.tile([rpw, nwin], F32, tag="ostage")

            # ---- dinv = 1/sqrt(deg) ----
            deg_sb = const.tile([P, nwin], F32, tag="deg")
            nc.sync.dma_start(deg_sb[:], deg_in[:])
            sq_sb = const.tile([P, nwin], F32, tag="sq")
            nc.scalar.activation(sq_sb[:], deg_sb[:],
                                 mybir.ActivationFunctionType.Sqrt)
            dinv = const.tile([P, nwin], F32, tag="dinv")
            nc.vector.reciprocal(dinv[:], sq_sb[:])

            # round-robin SWDGE queue assignment for gathers
            qctr = [0]

            def next_queue():
                q = qctr[0] % 4
                qctr[0] += 1
                return q

            # ---- xs = bf16(dinv * x) -> quarter shards, quarter AllGathers
            with tc.tile_pool(name="xsp", bufs=3) as xsp:
                nw_blk = 4
                for w0 in range(0, nwin, nw_blk):
                    nb = min(nw_blk, nwin - w0)
                    xt = xsp.tile([P, nb, d_in], F32, tag="xt")
                    nc.sync.dma_start(
                        xt[:],
                        x_in.ap().rearrange("(w p) d -> p w d", p=P)[:, w0:w0 + nb, :])
                    xs_t = xsp.tile([P, nb, d_in], MBF16, tag="xst")
                    nc.vector.tensor_tensor(
                        out=xs_t[:], in0=xt[:],
                        in1=dinv[:, w0:w0 + nb, None].to_broadcast([P, nb, d_in]),
                        op=mybir.AluOpType.mult)
                    q = w0 // wq
                    nc.sync.dma_start(
                        xs_sh4[q][:].rearrange("(w p) d -> p w d", p=P)
                        [:, (w0 % wq):(w0 % wq) + nb, :],
                        xs_t[:])
                    if (w0 + nb) % wq == 0:
                        nc.gpsimd.collective_compute(
                            "AllGather", mybir.AluOpType.bypass,
                            replica_groups=rg,
                            ins=[xs_sh4[q].opt()], outs=[xs_q[q].opt()])

            # ---- pools ----
            # PSUM: part1(3) + part2(3) + tp(1) + dn(1) = 8 banks
            alloc_order = []

            def apool(**kw):
                p = tc.alloc_tile_pool(**kw)
                alloc_order.append(p)
                return p

            tp_pool = apool(name="tpp", bufs=1, space="PSUM")
            dn_pool = apool(name="dnp", bufs=1, space="PSUM")

            def mk_layer_pools(tag, slab_bufs):
                return dict(
                    nslab=slab_bufs,
                    slab=apool(name=f"slab{tag}", bufs=slab_bufs),
                    idx=apool(name=f"idxp{tag}", bufs=8),
                    dst=apool(name=f"dstp{tag}", bufs=2),
                    sel=apool(name=f"selp{tag}", bufs=6),
                    part=apool(name=f"part{tag}", bufs=3, space="PSUM"),
                    acc=apool(name=f"accp{tag}", bufs=2 * G * SGG + 3),
                    own=apool(name=f"ownp{tag}", bufs=4),
                    epi=apool(name=f"epip{tag}", bufs=4),
                    tp=tp_pool,
                    dn=dn_pool,
                )

            pools1 = mk_layer_pools("1", 8)
            pools2 = mk_layer_pools("2", 7)

            def layer(d, tables, own4, epilogue, pp, quarter_cb=None):
                """aggregate (A01+I) @ table rows per dst window, then epilogue.

                Supergroups of SGG groups iterate chunk-major; each (group,
                chunk) is one dma_gather on its own SWDGE queue. Partials go
                PSUM -> SBUF accumulator (DVE add) so PSUM never serializes.
                """
                # prime slab buffers once (full extent) so rows skipped by
                # trailing -1 indices read 0.0 instead of NaN garbage
                for _ in range(pp["nslab"]):
                    z = pp["slab"].tile([P, maxblk, d], MBF16, tag="slab")
                    nc.vector.memset(z[:], 0)

                for t in range(nsg):
                    groups = range(t * SGG, min((t + 1) * SGG, ngroups))
                    wins = [w for g in groups
                            for w in range(g * G, min((g + 1) * G, nwin))]
                    c0 = min(dst_off[(w, ch)] for w in wins
                             for ch in range(nchunks) if S[w, ch] > 0)
                    c1 = max(dst_off[(w, ch)] + S[w, ch] // P for w in wins
                             for ch in range(nchunks) if S[w, ch] > 0)
                    dst_t = pp["dst"].tile([P, c1 - c0], MBF16, tag="dst")
                    nc.sync.dma_start(dst_t[:], dstl_in[:, c0:c1])

                    # accumulator init = self-loop term: own table rows
                    accs = {}
                    for w in wins:
                        ow = pp["own"].tile([P, d], MBF16, tag="own")
                        qw = w // wq
                        r0 = (w % wq) * P
                        nc.sync.dma_start(ow[:], own4[qw][r0:r0 + P, :])
                        acc = pp["acc"].tile([P, d], F32, tag="acc",
                                             name=f"acc_{w}")
                        nc.scalar.copy(acc[:], ow[:])
                        accs[w] = acc

                    for ch in range(nchunks):
                        for g in groups:
                            hw = [w for w in range(g * G, min((g + 1) * G, nwin))
                                  if S[w, ch] > 0]
                            if not hw:
                                continue
                            sg = sum(int(S[w, ch]) for w in hw)
                            i0 = idx_off[(hw[0], ch)]
                            idx_t = pp["idx"].tile([P, sg // 16], I16, tag="idx")
                            nc.sync.dma_start(idx_t[:],
                                              idx_in[:, i0:i0 + sg // 16])
                            slab = pp["slab"].tile([P, sg // P, d], MBF16,
                                                   tag="slab")
                            nc.gpsimd.dma_gather(
                                out_ap=slab[:],
                                in_ap=tables[ch][:],
                                idxs_ap=idx_t[:],
                                num_idxs=sg, num_idxs_reg=sg, elem_size=d,
                                single_packet=False, queue_num=next_queue())
                            boff = 0
                            for w in hw:
                                s_wch = int(S[w, ch])
                                nb = s_wch // P
                                do = dst_off[(w, ch)] - c0
                                sel = pp["sel"].tile([P, nb, P], MBF16, tag="sel")
                                nc.vector.tensor_tensor(
                                    out=sel[:],
                                    in0=dst_t[:, do:do + nb, None]
                                    .to_broadcast([P, nb, P]),
                                    in1=iota_sb[:, None, :].to_broadcast([P, nb, P]),
                                    op=mybir.AluOpType.is_equal)
                                part = pp["part"].tile([P, d], F32, tag="part")
                                for b in range(nb):
                                    nc.tensor.matmul(
                                        part[:], lhsT=sel[:, b, :],
                                        rhs=slab[:, boff + b, :],
                                        start=b == 0, stop=b == nb - 1)
                                boff += nb
                                nc.vector.tensor_tensor(
                                    out=accs[w][:], in0=accs[w][:], in1=part[:],
                                    op=mybir.AluOpType.add)
                    for w in wins:
                        epilogue(w, accs[w], pp)
                    if quarter_cb is not None and (t + 1) % sgq == 0:
                        quarter_cb((t + 1) // sgq - 1)

            def epi1(w, acc, pp):
                t = pp["epi"].tile([P, d_in], F32, tag="t1")
                nc.scalar.activation(t[:], acc[:],
                                     mybir.ActivationFunctionType.Copy,
                                     scale=dinv[:, w:w + 1])
                h_ps = pp["dn"].tile([P, d_hid], F32, tag="dn")
                for k in range(d_in // P):
                    tp = pp["tp"].tile([P, P], F32, tag="tp")
                    nc.tensor.transpose(
                        out=tp[:], in_=t[:, k * P:(k + 1) * P], identity=identity[:])
                    tT = pp["epi"].tile([P, P], F32, tag="tT1")
                    nc.scalar.copy(tT[:], tp[:])
                    nc.tensor.matmul(h_ps[:], lhsT=tT[:], rhs=w1_sb[:],
                                     start=k == 0, stop=k == d_in // P - 1)
                v = pp["epi"].tile([P, d_hid], F32, tag="v1")
                nc.vector.tensor_tensor(out=v[:], in0=h_ps[:], in1=b1rep[:],
                                        op=mybir.AluOpType.add)
                # g1 = dinv*relu(v) == relu(dinv*v) since dinv > 0
                g1w = pp["epi"].tile([P, d_hid], MBF16, tag="g1w")
                nc.scalar.activation(g1w[:], v[:],
                                     mybir.ActivationFunctionType.Relu,
                                     scale=dinv[:, w:w + 1])
                q = w // wq
                r0 = (w % wq) * P
                nc.sync.dma_start(g1_sh4[q][r0:r0 + P, :], g1w[:])

            def epi2(w, acc, pp):
                t = pp["epi"].tile([P, d_hid], F32, tag="t2")
                nc.scalar.activation(t[:], acc[:],
                                     mybir.ActivationFunctionType.Copy,
                                     scale=dinv[:, w:w + 1])
                h_ps = pp["dn"].tile([P, d_hid], F32, tag="dn")
                for k in range(d_hid // P):
                    tp = pp["tp"].tile([P, P], F32, tag="tp")
                    nc.tensor.transpose(
                        out=tp[:], in_=t[:, k * P:(k + 1) * P], identity=identity[:])
                    tT = pp["epi"].tile([P, P], F32, tag="tT2")
                    nc.scalar.copy(tT[:], tp[:])
                    nc.tensor.matmul(h_ps[:], lhsT=tT[:], rhs=w2_sb[k][:],
                                     start=k == 0, stop=k == d_hid // P - 1)
                v = pp["epi"].tile([P, d_hid], F32, tag="v2")
                nc.vector.tensor_tensor(out=v[:], in0=h_ps[:], in1=b2rep[:],
                                        op=mybir.AluOpType.add)
                r = pp["epi"].tile([P, d_hid], F32, tag="r2")
                nc.scalar.activation(r[:], v[:], mybir.ActivationFunctionType.Relu)
                pm = pp["epi"].tile([P, d_hid], F32, tag="pm")
                nc.vector.tensor_tensor(out=pm[:], in0=r[:], in1=wrep[:],
                                        op=mybir.AluOpType.mult)
                z_ps = pp["dn"].tile([rpw, d_hid], F32, tag="dn")
                nc.tensor.matmul(z_ps[:], lhsT=u_sb[:], rhs=pm[:],
                                 start=True, stop=True)
                nc.vector.reduce_sum(out=out_stage[:, w:w + 1], in_=z_ps[:],
                                     axis=mybir.AxisListType.X)

            def g1_ag(q):
                nc.gpsimd.collective_compute(
                    "AllGather", mybir.AluOpType.bypass,
                    replica_groups=rg,
                    ins=[g1_sh4[q].opt()], outs=[g1_q[q].opt()])

            layer(d_in, xs_q, xs_sh4, epi1, pools1, quarter_cb=g1_ag)
            layer(d_hid, g1_q, g1_sh4, epi2, pools2)

            # ---- finalize output ----
            out_f = const.tile([rpw, nwin], F32, tag="outf")
            nc.vector.tensor_tensor(out=out_f[:], in0=out_stage[:],
                                    in1=boutrep[:].to_broadcast([rpw, nwin]),
                                    op=mybir.AluOpType.add)
            nc.sync.dma_start(
                out.ap().rearrange("(w r) one -> r (w one)", r=rpw), out_f[:])

            for pool in reversed(alloc_order):
                pool.release()

    nc.compile()
    return nc


def _run(inputs, cfg, trace=False):
    x = np.asarray(inputs["x"], dtype=np.float32)
    edge_index = np.asarray(inputs["edge_index"])
    W1 = np.asarray(inputs["W1"], dtype=np.float32)
    b1 = np.asarray(inputs["b1"], dtype=np.float32)
    W2 = np.asarray(inputs["W2"], dtype=np.float32)
    b2 = np.asarray(inputs["b2"], dtype=np.float32)
    Wout = np.asarray(inputs["Wout"], dtype=np.float32)
    bout = np.asarray(inputs["bout"], dtype=np.float32)

    n, ncores, sub = cfg["n"], cfg["ncores"], cfg["sub"]
    d_in, d_hid = cfg["d_in"], cfg["d_hid"]
    ns = n // ncores
    rpw = P // sub

    meta, idx_up, dst_up, deg_up = _prep(x, edge_index, cfg)
    nc = _build(cfg, meta)

    woutr = Wout.reshape(sub, d_hid)
    u = np.zeros((P, rpw), dtype=np.float32)
    u[np.arange(P), np.arange(P) // sub] = 1.0
    iota = np.tile(np.arange(P, dtype=np.float32), (P, 1)).astype(BF16)

    in_maps = []
    for c in range(ncores):
        in_maps.append({
            "x": np.ascontiguousarray(x[c * ns:(c + 1) * ns]),
            "deg": np.ascontiguousarray(deg_up[c]),
            "W1": W1, "b1": b1, "W2": W2, "b2": b2,
            "WoutR": np.ascontiguousarray(woutr), "bout": bout,
            "U": u, "iota": iota,
            "idx": np.ascontiguousarray(idx_up[c]),
            "dstl": np.ascontiguousarray(dst_up[c]),
        })
    res = bass_utils.run_bass_kernel_spmd(
        nc, in_maps, core_ids=list(range(ncores)), trace=trace)
    outp = np.concatenate([res.results[c]["out"] for c in range(ncores)], axis=0)
    return outp, res


def kernel(**inputs):
    out, _ = _run(inputs, _cfg_full(), trace=False)
    return out
